# revision 23
# baseline (speedup 1.0000x reference)
"""CARAFE forward on 8 Trainium2 NeuronCores, data-parallel over batch.

Per core (1 sample):
  1. x loaded as 4 big contiguous DMAs, cast to fp16 padded layout x16
     ([*, 66, 68], center window 4B-aligned) plus one wide shifted copy
     xbf_w ([*, 66, 66]) whose kj=0 / kj=2 windows are both 4B-aligned,
     so every DVE multiply runs in 2x (16-bit packed) mode.
  2. 1x1 conv compressor (PE fp16); BN batch-stat sums fused into the psum
     evacuation (scalar accum_out), sumsq via a Square pass.
  3. BN stats AllReduce over the 8 cores (exact sync-BN), BN+ReLU applied.
  4. 3x3 encoder conv (PE, 9 taps psum-accumulated), fused bias+exp evac.
  5. softmax over H: strided reduce + reciprocal + fp16 normalize; kern
     written to DRAM so it can be partition-broadcast.
  6. reassembly, s-outer: per (s,t) ONE full-row broadcast [128, 4096]
     (8KB descriptors); DVE fp16 multiplies; tap accumulation on the PE
     via perm-matmuls (perm applies the faithful-to-source (s,c) reshape
     scramble: psum partition p' = (c%4)*32 + (c//4)%32, so each
     32-partition psum block is one output quadrant); scalar evacuates
     the 4 quadrant blocks into a [32, 2HQ, 128] staging tile that maps
     to a fully-contiguous 16KB-per-channel output DMA.
"""
import numpy as np

import concourse.bass as bass
import concourse.tile as tile
from concourse import bacc, mybir
from concourse.bass_utils import run_bass_kernel_spmd
from concourse.masks import make_identity

F32 = mybir.dt.float32
F16 = mybir.dt.float16
AX = mybir.AxisListType
OP = mybir.AluOpType
AF = mybir.ActivationFunctionType

B, C, H, W = 8, 256, 64, 64
CC = 64          # compressed channels
S = 2            # scale factor
K = 3            # kernel size
E = S * S * K * K  # 36 encoder channels
EPS = 1e-5
NCORES = 8
HP = H + 2       # padded rows
WP16 = 68        # x16 padded row width (col 0 unused, cols 1..66 = pad,x,pad)
NPIX = H * W
HQ = 16          # h rows per reassembly unit
NQ = H // HQ     # 4 units per (s)


def _ap(t, ap, extra_offset=0):
    return bass.AP(tensor=t.tensor, offset=t.offset + extra_offset, ap=ap)


def build():
    nc = bacc.Bacc("TRN2", target_bir_lowering=False, debug=False,
                   num_devices=NCORES)
    x_d = nc.dram_tensor("x", [C, H, W], F32, kind="ExternalInput").ap()
    w1_d = nc.dram_tensor("w1", [CC, C], F32, kind="ExternalInput").ap()
    b1_d = nc.dram_tensor("b1", [CC, 1], F32, kind="ExternalInput").ap()
    gamma_d = nc.dram_tensor("gamma", [CC, 1], F32, kind="ExternalInput").ap()
    beta_d = nc.dram_tensor("beta", [CC, 1], F32, kind="ExternalInput").ap()
    w2_d = nc.dram_tensor("w2", [E, CC * K * K], F32, kind="ExternalInput").ap()
    b2_d = nc.dram_tensor("b2", [E, 1], F32, kind="ExternalInput").ap()
    # perm[c, p'] = 1 iff c == 4*(p' % 32) + p' // 32
    perm_d = nc.dram_tensor("perm", [128, 128], F32, kind="ExternalInput").ap()
    out_d = nc.dram_tensor("out", [C, S * H, S * W], F32, kind="ExternalOutput").ap()

    with tile.TileContext(nc) as tc:
        with (
            tc.tile_pool(name="persist", bufs=1) as persist,
            tc.tile_pool(name="dram", bufs=1, space="DRAM") as dram,
        ):
            # ---------- constants & transposed weights ----------
            ident = persist.tile([128, 128], F32)
            make_identity(nc, ident)
            perm_sb = persist.tile([128, 128], F32)
            nc.sync.dma_start(out=perm_sb, in_=perm_d)
            perm16 = persist.tile([128, 128], F16)
            nc.scalar.copy(out=perm16, in_=perm_sb)

            w1T = persist.tile([128, 2, CC], F16)   # (c_part, chunk, o)
            w2T = persist.tile([CC, K * K, E], F16)  # (c, tap, e)
            b1_sb = persist.tile([CC, 1], F32)
            nc.sync.dma_start(out=b1_sb, in_=b1_d)
            gamma_sb = persist.tile([CC, 1], F32)
            nc.sync.dma_start(out=gamma_sb, in_=gamma_d)
            beta_sb = persist.tile([CC, 1], F32)
            nc.sync.dma_start(out=beta_sb, in_=beta_d)
            b2_sb = persist.tile([E, 1], F32)
            nc.sync.dma_start(out=b2_sb, in_=b2_d)

            with (
                tc.tile_pool(name="wld", bufs=1) as wld,
                tc.tile_pool(name="tp", bufs=2, space="PSUM") as tps,
            ):
                w1_sb = wld.tile([CC, C], F32)
                nc.sync.dma_start(out=w1_sb, in_=w1_d)
                w2_sb = wld.tile([E, CC * K * K], F32)
                nc.sync.dma_start(out=w2_sb, in_=w2_d)
                for ck in range(2):
                    pt = tps.tile([128, CC], F32, tag="w1t")
                    nc.tensor.transpose(pt, w1_sb[:, ck * 128:(ck + 1) * 128],
                                        ident[:CC, :CC])
                    nc.scalar.copy(out=w1T[:, ck, :], in_=pt)
                for t in range(K * K):
                    pt2 = tps.tile([CC, E], F32, tag="w2t")
                    src = _ap(w2_sb[:, :], [w2_sb[:, :].ap[0], [K * K, CC]],
                              extra_offset=t)
                    nc.tensor.transpose(pt2, src, ident[:E, :E])
                    nc.scalar.copy(out=w2T[:, t, :], in_=pt2)

            # ---------- x: load, cast to padded fp16, wide shifted copy ----
            x16 = persist.tile([128, 2, HP, WP16], F16)
            xbf = persist.tile([128, 2, HP, HP], F16)  # = x16 cols 1..66
            NLD = 32
            with tc.tile_pool(name="xfp", bufs=2) as xfp:
                for ck in range(2):
                    for r0 in range(0, H, NLD):
                        xf = xfp.tile([128, NLD, W], F32, tag="xload")
                        nc.sync.dma_start(
                            out=xf, in_=x_d[ck * 128:(ck + 1) * 128,
                                            r0:r0 + NLD, :])
                        nc.gpsimd.tensor_copy(
                            out=x16[:, ck, 1 + r0:1 + r0 + NLD, 2:2 + W],
                            in_=xf)
                for ck in range(2):
                    nc.vector.tensor_copy(out=x16[:, ck, 1:H + 1, 1:2],
                                          in_=x16[:, ck, 1:H + 1, 2:3])
                    nc.vector.tensor_copy(out=x16[:, ck, 1:H + 1, 66:67],
                                          in_=x16[:, ck, 1:H + 1, 65:66])
                    nc.vector.tensor_copy(out=x16[:, ck, 0:1, 1:67],
                                          in_=x16[:, ck, 1:2, 1:67])
                    nc.vector.tensor_copy(out=x16[:, ck, HP - 1:HP, 1:67],
                                          in_=x16[:, ck, HP - 2:HP - 1, 1:67])
                for ck in range(2):
                    eng = nc.scalar.copy if ck == 0 else nc.gpsimd.tensor_copy
                    eng(out=xbf[:, ck, :, :], in_=x16[:, ck, :, 1:1 + HP])

            # ---------- compressor + BN + encoder + softmax (scoped) -------
            kern_dr = dram.tile([E, H * W], F16)
            with tc.tile_pool(name="mid", bufs=1) as mid:
                comp = mid.tile([CC, HP, HP], F16)
                nc.vector.memset(comp[:, 0:1, :], 0.0)
                nc.vector.memset(comp[:, HP - 1:HP, :], 0.0)
                nc.vector.memset(comp[:, :, 0:1], 0.0)
                nc.vector.memset(comp[:, :, HP - 1:HP], 0.0)
                NCH = 8
                nchunks = H // NCH
                sum_p = mid.tile([CC, nchunks], F32)
                sq_p = mid.tile([CC, nchunks], F32)
                dump = mid.tile([CC, NCH * W], F16)
                with tc.tile_pool(name="cps", bufs=2, space="PSUM") as cps:
                    for hc in range(nchunks):
                        pc = cps.tile([CC, NCH, W], F32, tag="comp")
                        for ck in range(2):
                            nc.tensor.matmul(
                                pc, w1T[:, ck, :],
                                x16[:, ck, 1 + hc * NCH:1 + (hc + 1) * NCH,
                                    2:2 + W],
                                start=(ck == 0), stop=(ck == 1))
                        nc.scalar.activation(
                            out=comp[:, 1 + hc * NCH:1 + (hc + 1) * NCH,
                                     1:W + 1],
                            in_=pc, func=AF.Identity, bias=b1_sb, scale=1.0,
                            accum_out=sum_p[:, hc:hc + 1])
                    for hc in range(nchunks):
                        nc.scalar.activation(
                            out=dump.rearrange("p (a b) -> p a b", a=NCH),
                            in_=comp[:, 1 + hc * NCH:1 + (hc + 1) * NCH,
                                     1:W + 1],
                            func=AF.Square, accum_out=sq_p[:, hc:hc + 1])

                stats = mid.tile([CC, 2], F32)
                nc.vector.tensor_reduce(out=stats[:, 0:1], in_=sum_p,
                                        axis=AX.X, op=OP.add)
                nc.vector.tensor_reduce(out=stats[:, 1:2], in_=sq_p,
                                        axis=AX.X, op=OP.add)

                cc_in = dram.tile([CC, 2], F32)
                cc_out = dram.tile([CC, 2], F32)
                nc.gpsimd.dma_start(out=cc_in[:], in_=stats)
                nc.gpsimd.collective_compute(
                    "AllReduce", OP.add,
                    replica_groups=[list(range(NCORES))],
                    ins=[cc_in[:].opt()], outs=[cc_out[:].opt()])
                gstats = mid.tile([CC, 2], F32)
                nc.gpsimd.dma_start(out=gstats, in_=cc_out[:])

                mu = mid.tile([CC, 1], F32)
                var = mid.tile([CC, 1], F32)
                scl = mid.tile([CC, 1], F32)
                shf = mid.tile([CC, 1], F32)
                inv_n = 1.0 / (B * NPIX)
                nc.vector.tensor_scalar_mul(out=mu, in0=gstats[:, 0:1],
                                            scalar1=inv_n)
                nc.vector.tensor_scalar_mul(out=var, in0=gstats[:, 1:2],
                                            scalar1=inv_n)
                nc.vector.tensor_tensor(out=shf, in0=mu, in1=mu, op=OP.mult)
                nc.vector.tensor_tensor(out=var, in0=var, in1=shf,
                                        op=OP.subtract)
                eps_sb = mid.tile([CC, 1], F32)
                nc.vector.memset(eps_sb, EPS)
                nc.scalar.activation(out=var, in_=var, func=AF.Sqrt,
                                     bias=eps_sb, scale=1.0)
                nc.vector.reciprocal(out=var, in_=var)
                nc.vector.tensor_tensor(out=scl, in0=gamma_sb, in1=var,
                                        op=OP.mult)
                nc.vector.tensor_tensor(out=shf, in0=mu, in1=scl, op=OP.mult)
                nc.vector.tensor_tensor(out=shf, in0=beta_sb, in1=shf,
                                        op=OP.subtract)
                interior = comp[:, 1:H + 1, 1:W + 1]
                nc.scalar.activation(out=interior, in_=interior, func=AF.Relu,
                                     bias=shf, scale=scl)

                eexp = mid.tile([E, H, W], F16)
                with tc.tile_pool(name="eps", bufs=2, space="PSUM") as eps_pool:
                    for hc in range(nchunks):
                        pe = eps_pool.tile([E, NCH, W], F32, tag="enc")
                        for t in range(K * K):
                            ki, kj = t // K, t % K
                            nc.tensor.matmul(
                                pe, w2T[:, t, :],
                                comp[:, hc * NCH + ki:hc * NCH + ki + NCH,
                                     kj:kj + W],
                                start=(t == 0), stop=(t == K * K - 1))
                        nc.scalar.activation(
                            out=eexp[:, hc * NCH:(hc + 1) * NCH, :], in_=pe,
                            func=AF.Exp, bias=b2_sb, scale=1.0)

                zrec = mid.tile([E, W], F32)
                ee = eexp[:, :, :]
                ee_wh = _ap(ee, [ee.ap[0], [1, W], [W, H]])
                nc.vector.tensor_reduce(out=zrec, in_=ee_wh, axis=AX.X,
                                        op=OP.add)
                nc.vector.reciprocal(out=zrec, in_=zrec)
                zrec16 = mid.tile([E, W], F16)
                nc.vector.tensor_copy(out=zrec16, in_=zrec)
                kern = mid.tile([E, H, W], F16)
                zb = zrec16[:, :]
                nc.vector.tensor_tensor(
                    out=kern, in0=ee,
                    in1=_ap(zb, [zb.ap[0], [0, H], [1, W]]), op=OP.mult)
                nc.gpsimd.dma_start(out=kern_dr[:],
                                    in_=kern.rearrange("p a b -> p (a b)"))

            # ---------- reassembly (s-outer, full-row broadcasts) ----------
            with (
                tc.tile_pool(name="bc", bufs=9) as bcpool,
                tc.tile_pool(name="prod", bufs=6) as prodpool,
                tc.tile_pool(name="stg", bufs=3) as stgpool,
                tc.tile_pool(name="ops", bufs=2, space="PSUM") as pspool,
            ):
                for s in range(S * S):
                    bcs = []
                    for t in range(K * K):
                        ch = s * K * K + t
                        bc = bcpool.tile([128, H * W], F16, tag="bc",
                                         name=f"bc_{s}_{t}")
                        eng = (nc.gpsimd, nc.sync, nc.scalar)[t % 3]
                        eng.dma_start(
                            out=bc,
                            in_=_ap(kern_dr[ch:ch + 1, :],
                                    [[0, 128], [1, H * W]]))
                        bcs.append(bc)
                    for q in range(NQ):
                        h0 = q * HQ
                        for ck in range(2):
                            psum = pspool.tile(
                                [128, HQ * W], F32, tag="ps", bufs=4,
                                name=f"ps_{s}_{q}_{ck}")
                            for t in range(K * K):
                                ki, kj = t // K, t % K
                                in1 = _ap(bcs[t][:, :],
                                          [bcs[t][:, :].ap[0], [W, HQ],
                                           [1, W]],
                                          extra_offset=h0 * W)
                                prod = prodpool.tile([128, HQ, W], F16)
                                if kj == 1:
                                    in0 = x16[:, ck, h0 + ki:h0 + ki + HQ,
                                              2:2 + W]
                                else:
                                    in0 = xbf[:, ck, h0 + ki:h0 + ki + HQ,
                                              kj:kj + W]
                                nc.vector.tensor_tensor(
                                    out=prod, in0=in0, in1=in1, op=OP.mult)
                                prod_f = prod.rearrange("p a b -> p (a b)")
                                for i in range(HQ * W // 512):
                                    nc.tensor.matmul(
                                        psum[:, i * 512:(i + 1) * 512],
                                        perm16,
                                        prod_f[:, i * 512:(i + 1) * 512],
                                        start=(t == 0), stop=(t == K * K - 1))
                            # evac: psum block kap = quadrant (hb, wb); value
                            # (p'=kap*32+chi, hl, w) ->
                            #   olin[chi, 2*hl+hb, wb*64+w]
                            olin = stgpool.tile([32, 2 * HQ, S * W], F32,
                                                tag="ol",
                                                name=f"olin_{s}_{q}_{ck}")
                            ob = olin[:, :, :]
                            for kap in range(4):
                                hb, wb = kap // 2, kap % 2
                                dst_view = _ap(
                                    ob, [ob.ap[0], [2 * S * W, HQ], [1, W]],
                                    extra_offset=hb * S * W + wb * W)
                                sl = psum[kap * 32:(kap + 1) * 32, :]
                                src_view = _ap(sl,
                                               [sl.ap[0], [W, HQ], [1, W]])
                                nc.scalar.copy(out=dst_view, in_=src_view)
                            dst = bass.AP(
                                tensor=out_d.tensor,
                                offset=out_d.offset
                                + (s * 64 + ck * 32) * (S * H * S * W)
                                + (2 * h0) * (S * W),
                                ap=[[S * H * S * W, 32],
                                    [S * W, 2 * HQ],
                                    [1, S * W]])
                            nc.sync.dma_start(out=dst, in_=olin)
    nc.compile()
    return nc


_NC_CACHE = None


def _get_nc():
    global _NC_CACHE
    if _NC_CACHE is None:
        _NC_CACHE = build()
    return _NC_CACHE


def _perm_matrix():
    p = np.zeros((128, 128), dtype=np.float32)
    for pp in range(128):
        c = 4 * (pp % 32) + pp // 32
        p[c, pp] = 1.0
    return p


def _make_in_maps(inputs):
    x = np.ascontiguousarray(inputs["x"], dtype=np.float32)
    perm = _perm_matrix()
    in_maps = []
    for b in range(NCORES):
        in_maps.append({
            "perm": perm,
            "x": np.ascontiguousarray(x[b]),
            "w1": np.ascontiguousarray(inputs["w1"], dtype=np.float32),
            "b1": np.ascontiguousarray(np.asarray(inputs["b1"], dtype=np.float32).reshape(CC, 1)),
            "gamma": np.ascontiguousarray(np.asarray(inputs["gamma"], dtype=np.float32).reshape(CC, 1)),
            "beta": np.ascontiguousarray(np.asarray(inputs["beta"], dtype=np.float32).reshape(CC, 1)),
            "w2": np.ascontiguousarray(np.asarray(inputs["w2"], dtype=np.float32).reshape(E, CC * K * K)),
            "b2": np.ascontiguousarray(np.asarray(inputs["b2"], dtype=np.float32).reshape(E, 1)),
        })
    return in_maps


def kernel(x, w1, b1, gamma, beta, w2, b2, **kwargs):
    in_maps = _make_in_maps(dict(x=x, w1=w1, b1=b1, gamma=gamma, beta=beta,
                                 w2=w2, b2=b2))
    nc = _get_nc()
    res = run_bass_kernel_spmd(nc, in_maps, core_ids=list(range(NCORES)))
    return np.stack([res.results[b]["out"] for b in range(NCORES)], axis=0)


# revision 24
# speedup vs baseline: 1.0730x; 1.0730x over previous
"""CARAFE forward on 8 Trainium2 NeuronCores, data-parallel over batch.

Per core (1 sample):
  1. x loaded as 4 big contiguous DMAs, cast to fp16 padded layout x16
     ([*, 66, 68], center window 4B-aligned) plus one wide shifted copy
     xbf_w ([*, 66, 66]) whose kj=0 / kj=2 windows are both 4B-aligned,
     so every DVE multiply runs in 2x (16-bit packed) mode.
  2. 1x1 conv compressor (PE fp16); BN batch-stat sums fused into the psum
     evacuation (scalar accum_out), sumsq via a Square pass.
  3. BN stats AllReduce over the 8 cores (exact sync-BN), BN+ReLU applied.
  4. 3x3 encoder conv (PE, 9 taps psum-accumulated), fused bias+exp evac.
  5. softmax over H: strided reduce + reciprocal + fp16 normalize; kern
     written to DRAM so it can be partition-broadcast.
  6. reassembly, s-outer: per (s,t) ONE full-row broadcast [128, 4096]
     (8KB descriptors); DVE fp16 multiplies; tap accumulation on the PE
     via perm-matmuls (perm applies the faithful-to-source (s,c) reshape
     scramble: psum partition p' = (c%4)*32 + (c//4)%32, so each
     32-partition psum block is one output quadrant); scalar evacuates
     the 4 quadrant blocks into a [32, 2HQ, 128] staging tile that maps
     to a fully-contiguous 16KB-per-channel output DMA.
"""
import numpy as np

import concourse.bass as bass
import concourse.tile as tile
from concourse import bacc, mybir
from concourse.bass_utils import run_bass_kernel_spmd
from concourse.masks import make_identity

F32 = mybir.dt.float32
F16 = mybir.dt.float16
AX = mybir.AxisListType
OP = mybir.AluOpType
AF = mybir.ActivationFunctionType

B, C, H, W = 8, 256, 64, 64
CC = 64          # compressed channels
S = 2            # scale factor
K = 3            # kernel size
E = S * S * K * K  # 36 encoder channels
EPS = 1e-5
NCORES = 8
HP = H + 2       # padded rows
WP16 = 68        # x16 padded row width (col 0 unused, cols 1..66 = pad,x,pad)
NPIX = H * W
HQ = 16          # h rows per reassembly unit
NQ = H // HQ     # 4 units per (s)


def _ap(t, ap, extra_offset=0):
    return bass.AP(tensor=t.tensor, offset=t.offset + extra_offset, ap=ap)


def build():
    nc = bacc.Bacc("TRN2", target_bir_lowering=False, debug=False,
                   num_devices=NCORES)
    x_d = nc.dram_tensor("x", [C, H, W], F32, kind="ExternalInput").ap()
    w1_d = nc.dram_tensor("w1", [CC, C], F32, kind="ExternalInput").ap()
    b1_d = nc.dram_tensor("b1", [CC, 1], F32, kind="ExternalInput").ap()
    gamma_d = nc.dram_tensor("gamma", [CC, 1], F32, kind="ExternalInput").ap()
    beta_d = nc.dram_tensor("beta", [CC, 1], F32, kind="ExternalInput").ap()
    w2_d = nc.dram_tensor("w2", [E, CC * K * K], F32, kind="ExternalInput").ap()
    b2_d = nc.dram_tensor("b2", [E, 1], F32, kind="ExternalInput").ap()
    # perm[c, p'] = 1 iff c == 4*(p' % 32) + p' // 32
    perm_d = nc.dram_tensor("perm", [128, 128], F32, kind="ExternalInput").ap()
    out_d = nc.dram_tensor("out", [C, S * H, S * W], F32, kind="ExternalOutput").ap()

    with tile.TileContext(nc) as tc:
        with (
            tc.tile_pool(name="persist", bufs=1) as persist,
            tc.tile_pool(name="dram", bufs=1, space="DRAM") as dram,
        ):
            # ---------- constants & transposed weights ----------
            ident = persist.tile([128, 128], F32)
            make_identity(nc, ident)
            perm_sb = persist.tile([128, 128], F32)
            nc.sync.dma_start(out=perm_sb, in_=perm_d)
            perm16 = persist.tile([128, 128], F16)
            nc.scalar.copy(out=perm16, in_=perm_sb)

            w1T = persist.tile([128, 2, CC], F16)   # (c_part, chunk, o)
            w2T = persist.tile([CC, K * K, E], F16)  # (c, tap, e)
            b1_sb = persist.tile([CC, 1], F32)
            nc.sync.dma_start(out=b1_sb, in_=b1_d)
            gamma_sb = persist.tile([CC, 1], F32)
            nc.sync.dma_start(out=gamma_sb, in_=gamma_d)
            beta_sb = persist.tile([CC, 1], F32)
            nc.sync.dma_start(out=beta_sb, in_=beta_d)
            b2_sb = persist.tile([E, 1], F32)
            nc.sync.dma_start(out=b2_sb, in_=b2_d)

            with (
                tc.tile_pool(name="wld", bufs=1) as wld,
                tc.tile_pool(name="tp", bufs=2, space="PSUM") as tps,
            ):
                w1_sb = wld.tile([CC, C], F32)
                nc.sync.dma_start(out=w1_sb, in_=w1_d)
                w2_sb = wld.tile([E, CC * K * K], F32)
                nc.sync.dma_start(out=w2_sb, in_=w2_d)
                for ck in range(2):
                    pt = tps.tile([128, CC], F32, tag="w1t")
                    nc.tensor.transpose(pt, w1_sb[:, ck * 128:(ck + 1) * 128],
                                        ident[:CC, :CC])
                    nc.scalar.copy(out=w1T[:, ck, :], in_=pt)
                for t in range(K * K):
                    pt2 = tps.tile([CC, E], F32, tag="w2t")
                    src = _ap(w2_sb[:, :], [w2_sb[:, :].ap[0], [K * K, CC]],
                              extra_offset=t)
                    nc.tensor.transpose(pt2, src, ident[:E, :E])
                    nc.scalar.copy(out=w2T[:, t, :], in_=pt2)

            # ---------- x: load, cast to padded fp16, wide shifted copy ----
            x16 = persist.tile([128, 2, HP, WP16], F16)
            xbf = persist.tile([128, 2, HP, HP], F16)  # = x16 cols 1..66
            NLD = 32
            with tc.tile_pool(name="xfp", bufs=2) as xfp:
                for ck in range(2):
                    for r0 in range(0, H, NLD):
                        xf = xfp.tile([128, NLD, W], F32, tag="xload")
                        nc.sync.dma_start(
                            out=xf, in_=x_d[ck * 128:(ck + 1) * 128,
                                            r0:r0 + NLD, :])
                        nc.gpsimd.tensor_copy(
                            out=x16[:, ck, 1 + r0:1 + r0 + NLD, 2:2 + W],
                            in_=xf)
                for ck in range(2):
                    nc.vector.tensor_copy(out=x16[:, ck, 1:H + 1, 1:2],
                                          in_=x16[:, ck, 1:H + 1, 2:3])
                    nc.vector.tensor_copy(out=x16[:, ck, 1:H + 1, 66:67],
                                          in_=x16[:, ck, 1:H + 1, 65:66])
                    nc.vector.tensor_copy(out=x16[:, ck, 0:1, 1:67],
                                          in_=x16[:, ck, 1:2, 1:67])
                    nc.vector.tensor_copy(out=x16[:, ck, HP - 1:HP, 1:67],
                                          in_=x16[:, ck, HP - 2:HP - 1, 1:67])
                for ck in range(2):
                    eng = nc.scalar.copy if ck == 0 else nc.gpsimd.tensor_copy
                    eng(out=xbf[:, ck, :, :], in_=x16[:, ck, :, 1:1 + HP])

            # ---------- compressor + BN + encoder + softmax (scoped) -------
            kern_dr = dram.tile([E, H * W], F16)
            with tc.tile_pool(name="mid", bufs=1) as mid:
                comp = mid.tile([CC, HP, HP], F16)
                nc.vector.memset(comp[:, 0:1, :], 0.0)
                nc.vector.memset(comp[:, HP - 1:HP, :], 0.0)
                nc.vector.memset(comp[:, :, 0:1], 0.0)
                nc.vector.memset(comp[:, :, HP - 1:HP], 0.0)
                NCH = 8
                nchunks = H // NCH
                sum_p = mid.tile([CC, nchunks], F32)
                sq_p = mid.tile([CC, nchunks], F32)
                dump = mid.tile([CC, NCH * W], F16)
                with tc.tile_pool(name="cps", bufs=2, space="PSUM") as cps:
                    for hc in range(nchunks):
                        pc = cps.tile([CC, NCH, W], F32, tag="comp")
                        for ck in range(2):
                            nc.tensor.matmul(
                                pc, w1T[:, ck, :],
                                x16[:, ck, 1 + hc * NCH:1 + (hc + 1) * NCH,
                                    2:2 + W],
                                start=(ck == 0), stop=(ck == 1))
                        nc.scalar.activation(
                            out=comp[:, 1 + hc * NCH:1 + (hc + 1) * NCH,
                                     1:W + 1],
                            in_=pc, func=AF.Identity, bias=b1_sb, scale=1.0,
                            accum_out=sum_p[:, hc:hc + 1])
                    for hc in range(nchunks):
                        nc.scalar.activation(
                            out=dump.rearrange("p (a b) -> p a b", a=NCH),
                            in_=comp[:, 1 + hc * NCH:1 + (hc + 1) * NCH,
                                     1:W + 1],
                            func=AF.Square, accum_out=sq_p[:, hc:hc + 1])

                stats = mid.tile([CC, 2], F32)
                nc.vector.tensor_reduce(out=stats[:, 0:1], in_=sum_p,
                                        axis=AX.X, op=OP.add)
                nc.vector.tensor_reduce(out=stats[:, 1:2], in_=sq_p,
                                        axis=AX.X, op=OP.add)

                cc_in = dram.tile([CC, 2], F32)
                cc_out = dram.tile([CC, 2], F32)
                nc.gpsimd.dma_start(out=cc_in[:], in_=stats)
                nc.gpsimd.collective_compute(
                    "AllReduce", OP.add,
                    replica_groups=[list(range(NCORES))],
                    ins=[cc_in[:].opt()], outs=[cc_out[:].opt()])
                gstats = mid.tile([CC, 2], F32)
                nc.gpsimd.dma_start(out=gstats, in_=cc_out[:])

                mu = mid.tile([CC, 1], F32)
                var = mid.tile([CC, 1], F32)
                scl = mid.tile([CC, 1], F32)
                shf = mid.tile([CC, 1], F32)
                inv_n = 1.0 / (B * NPIX)
                nc.vector.tensor_scalar_mul(out=mu, in0=gstats[:, 0:1],
                                            scalar1=inv_n)
                nc.vector.tensor_scalar_mul(out=var, in0=gstats[:, 1:2],
                                            scalar1=inv_n)
                nc.vector.tensor_tensor(out=shf, in0=mu, in1=mu, op=OP.mult)
                nc.vector.tensor_tensor(out=var, in0=var, in1=shf,
                                        op=OP.subtract)
                eps_sb = mid.tile([CC, 1], F32)
                nc.vector.memset(eps_sb, EPS)
                nc.scalar.activation(out=var, in_=var, func=AF.Sqrt,
                                     bias=eps_sb, scale=1.0)
                nc.vector.reciprocal(out=var, in_=var)
                nc.vector.tensor_tensor(out=scl, in0=gamma_sb, in1=var,
                                        op=OP.mult)
                nc.vector.tensor_tensor(out=shf, in0=mu, in1=scl, op=OP.mult)
                nc.vector.tensor_tensor(out=shf, in0=beta_sb, in1=shf,
                                        op=OP.subtract)
                interior = comp[:, 1:H + 1, 1:W + 1]
                nc.scalar.activation(out=interior, in_=interior, func=AF.Relu,
                                     bias=shf, scale=scl)

                eexp = mid.tile([E, H, W], F16)
                with tc.tile_pool(name="eps", bufs=2, space="PSUM") as eps_pool:
                    for hc in range(nchunks):
                        pe = eps_pool.tile([E, NCH, W], F32, tag="enc")
                        for t in range(K * K):
                            ki, kj = t // K, t % K
                            nc.tensor.matmul(
                                pe, w2T[:, t, :],
                                comp[:, hc * NCH + ki:hc * NCH + ki + NCH,
                                     kj:kj + W],
                                start=(t == 0), stop=(t == K * K - 1))
                        nc.scalar.activation(
                            out=eexp[:, hc * NCH:(hc + 1) * NCH, :], in_=pe,
                            func=AF.Exp, bias=b2_sb, scale=1.0)

                zrec = mid.tile([E, W], F32)
                ee = eexp[:, :, :]
                ee_wh = _ap(ee, [ee.ap[0], [1, W], [W, H]])
                nc.vector.tensor_reduce(out=zrec, in_=ee_wh, axis=AX.X,
                                        op=OP.add)
                nc.vector.reciprocal(out=zrec, in_=zrec)
                zrec16 = mid.tile([E, W], F16)
                nc.vector.tensor_copy(out=zrec16, in_=zrec)
                kern = mid.tile([E, H, W], F16)
                zb = zrec16[:, :]
                nc.vector.tensor_tensor(
                    out=kern, in0=ee,
                    in1=_ap(zb, [zb.ap[0], [0, H], [1, W]]), op=OP.mult)
                nc.gpsimd.dma_start(out=kern_dr[:],
                                    in_=kern.rearrange("p a b -> p (a b)"))

            # ---------- reassembly (s-outer, full-row broadcasts) ----------
            with (
                tc.tile_pool(name="bc", bufs=9) as bcpool,
                tc.tile_pool(name="prod", bufs=6) as prodpool,
                tc.tile_pool(name="stg", bufs=3) as stgpool,
                tc.tile_pool(name="ops", bufs=2, space="PSUM") as pspool,
            ):
                for s in range(S * S):
                    bcs = []
                    for t in range(K * K):
                        ch = s * K * K + t
                        bc = bcpool.tile([128, H * W], F16, tag="bc",
                                         name=f"bc_{s}_{t}")
                        eng = (nc.gpsimd, nc.sync, nc.scalar)[t % 3]
                        eng.dma_start(
                            out=bc,
                            in_=_ap(kern_dr[ch:ch + 1, :],
                                    [[0, 128], [1, H * W]]))
                        bcs.append(bc)
                    for q in range(NQ):
                        h0 = q * HQ
                        psums = {}
                        for ck in range(2):
                            psums[ck] = pspool.tile(
                                [128, HQ * W], F32, tag=f"ps{ck}", bufs=2,
                                name=f"ps_{s}_{q}_{ck}")
                        for t in range(K * K):
                            ki, kj = t // K, t % K
                            in1 = _ap(bcs[t][:, :],
                                      [bcs[t][:, :].ap[0], [W, HQ], [1, W]],
                                      extra_offset=h0 * W)
                            for ck in range(2):
                                prod = prodpool.tile([128, HQ, W], F16)
                                if kj == 1:
                                    in0 = x16[:, ck, h0 + ki:h0 + ki + HQ,
                                              2:2 + W]
                                else:
                                    in0 = xbf[:, ck, h0 + ki:h0 + ki + HQ,
                                              kj:kj + W]
                                nc.vector.tensor_tensor(
                                    out=prod, in0=in0, in1=in1, op=OP.mult)
                                prod_f = prod.rearrange("p a b -> p (a b)")
                                for i in range(2):
                                    nc.tensor.matmul(
                                        psums[ck][:, i * 512:(i + 1) * 512],
                                        perm16,
                                        prod_f[:, i * 512:(i + 1) * 512],
                                        start=(t == 0), stop=(t == K * K - 1))
                        # evac: psum block kap = quadrant (hb, wb); value
                        # (p'=kap*32+chi, hl, w) -> olin[chi, 2*hl+hb, wb*64+w]
                        for ck in range(2):
                            olin = stgpool.tile([32, 2 * HQ, S * W], F32,
                                                tag="ol",
                                                name=f"olin_{s}_{q}_{ck}")
                            ob = olin[:, :, :]
                            for kap in range(4):
                                hb, wb = kap // 2, kap % 2
                                dst_view = _ap(
                                    ob, [ob.ap[0], [2 * S * W, HQ], [1, W]],
                                    extra_offset=hb * S * W + wb * W)
                                sl = psums[ck][kap * 32:(kap + 1) * 32, :]
                                src_view = _ap(sl,
                                               [sl.ap[0], [W, HQ], [1, W]])
                                nc.scalar.copy(out=dst_view, in_=src_view)
                            dst = bass.AP(
                                tensor=out_d.tensor,
                                offset=out_d.offset
                                + (s * 64 + ck * 32) * (S * H * S * W)
                                + (2 * h0) * (S * W),
                                ap=[[S * H * S * W, 32],
                                    [S * W, 2 * HQ],
                                    [1, S * W]])
                            nc.sync.dma_start(out=dst, in_=olin)
    nc.compile()
    return nc


_NC_CACHE = None


def _get_nc():
    global _NC_CACHE
    if _NC_CACHE is None:
        _NC_CACHE = build()
    return _NC_CACHE


def _perm_matrix():
    p = np.zeros((128, 128), dtype=np.float32)
    for pp in range(128):
        c = 4 * (pp % 32) + pp // 32
        p[c, pp] = 1.0
    return p


def _make_in_maps(inputs):
    x = np.ascontiguousarray(inputs["x"], dtype=np.float32)
    perm = _perm_matrix()
    in_maps = []
    for b in range(NCORES):
        in_maps.append({
            "perm": perm,
            "x": np.ascontiguousarray(x[b]),
            "w1": np.ascontiguousarray(inputs["w1"], dtype=np.float32),
            "b1": np.ascontiguousarray(np.asarray(inputs["b1"], dtype=np.float32).reshape(CC, 1)),
            "gamma": np.ascontiguousarray(np.asarray(inputs["gamma"], dtype=np.float32).reshape(CC, 1)),
            "beta": np.ascontiguousarray(np.asarray(inputs["beta"], dtype=np.float32).reshape(CC, 1)),
            "w2": np.ascontiguousarray(np.asarray(inputs["w2"], dtype=np.float32).reshape(E, CC * K * K)),
            "b2": np.ascontiguousarray(np.asarray(inputs["b2"], dtype=np.float32).reshape(E, 1)),
        })
    return in_maps


def kernel(x, w1, b1, gamma, beta, w2, b2, **kwargs):
    in_maps = _make_in_maps(dict(x=x, w1=w1, b1=b1, gamma=gamma, beta=beta,
                                 w2=w2, b2=b2))
    nc = _get_nc()
    res = run_bass_kernel_spmd(nc, in_maps, core_ids=list(range(NCORES)))
    return np.stack([res.results[b]["out"] for b in range(NCORES)], axis=0)


# revision 25
# speedup vs baseline: 1.0770x; 1.0037x over previous
"""CARAFE forward on 8 Trainium2 NeuronCores, data-parallel over batch.

Per core (1 sample):
  1. x loaded as 4 big contiguous DMAs, cast to fp16 padded layout x16
     ([*, 66, 68], center window 4B-aligned) plus one wide shifted copy
     xbf_w ([*, 66, 66]) whose kj=0 / kj=2 windows are both 4B-aligned,
     so every DVE multiply runs in 2x (16-bit packed) mode.
  2. 1x1 conv compressor (PE fp16); BN batch-stat sums fused into the psum
     evacuation (scalar accum_out), sumsq via a Square pass.
  3. BN stats AllReduce over the 8 cores (exact sync-BN), BN+ReLU applied.
  4. 3x3 encoder conv (PE, 9 taps psum-accumulated), fused bias+exp evac.
  5. softmax over H: strided reduce + reciprocal + fp16 normalize; kern
     written to DRAM so it can be partition-broadcast.
  6. reassembly, s-outer: per (s,t) ONE full-row broadcast [128, 4096]
     (8KB descriptors); DVE fp16 multiplies; tap accumulation on the PE
     via perm-matmuls (perm applies the faithful-to-source (s,c) reshape
     scramble: psum partition p' = (c%4)*32 + (c//4)%32, so each
     32-partition psum block is one output quadrant); scalar evacuates
     the 4 quadrant blocks into a [32, 2HQ, 128] staging tile that maps
     to a fully-contiguous 16KB-per-channel output DMA.
"""
import numpy as np

import concourse.bass as bass
import concourse.tile as tile
from concourse import bacc, mybir
from concourse.bass_utils import run_bass_kernel_spmd
from concourse.masks import make_identity

F32 = mybir.dt.float32
F16 = mybir.dt.float16
AX = mybir.AxisListType
OP = mybir.AluOpType
AF = mybir.ActivationFunctionType

B, C, H, W = 8, 256, 64, 64
CC = 64          # compressed channels
S = 2            # scale factor
K = 3            # kernel size
E = S * S * K * K  # 36 encoder channels
EPS = 1e-5
NCORES = 8
HP = H + 2       # padded rows
WP16 = 68        # x16 padded row width (col 0 unused, cols 1..66 = pad,x,pad)
NPIX = H * W
HQ = 16          # h rows per reassembly unit
NQ = H // HQ     # 4 units per (s)


def _ap(t, ap, extra_offset=0):
    return bass.AP(tensor=t.tensor, offset=t.offset + extra_offset, ap=ap)


def build():
    nc = bacc.Bacc("TRN2", target_bir_lowering=False, debug=False,
                   num_devices=NCORES)
    x_d = nc.dram_tensor("x", [C, H, W], F32, kind="ExternalInput").ap()
    w1_d = nc.dram_tensor("w1", [CC, C], F32, kind="ExternalInput").ap()
    b1_d = nc.dram_tensor("b1", [CC, 1], F32, kind="ExternalInput").ap()
    gamma_d = nc.dram_tensor("gamma", [CC, 1], F32, kind="ExternalInput").ap()
    beta_d = nc.dram_tensor("beta", [CC, 1], F32, kind="ExternalInput").ap()
    w2_d = nc.dram_tensor("w2", [E, CC * K * K], F32, kind="ExternalInput").ap()
    b2_d = nc.dram_tensor("b2", [E, 1], F32, kind="ExternalInput").ap()
    # perm[c, p'] = 1 iff c == 4*(p' % 32) + p' // 32
    perm_d = nc.dram_tensor("perm", [128, 128], F32, kind="ExternalInput").ap()
    out_d = nc.dram_tensor("out", [C, S * H, S * W], F32, kind="ExternalOutput").ap()

    with tile.TileContext(nc) as tc:
        with (
            tc.tile_pool(name="persist", bufs=1) as persist,
            tc.tile_pool(name="dram", bufs=1, space="DRAM") as dram,
        ):
            # ---------- constants & transposed weights ----------
            ident = persist.tile([128, 128], F32)
            make_identity(nc, ident)
            perm_sb = persist.tile([128, 128], F32)
            nc.sync.dma_start(out=perm_sb, in_=perm_d)
            perm16 = persist.tile([128, 128], F16)
            nc.scalar.copy(out=perm16, in_=perm_sb)

            w1T = persist.tile([128, 2, CC], F16)   # (c_part, chunk, o)
            w2T = persist.tile([CC, K * K, E], F16)  # (c, tap, e)
            b1_sb = persist.tile([CC, 1], F32)
            nc.sync.dma_start(out=b1_sb, in_=b1_d)
            gamma_sb = persist.tile([CC, 1], F32)
            nc.sync.dma_start(out=gamma_sb, in_=gamma_d)
            beta_sb = persist.tile([CC, 1], F32)
            nc.sync.dma_start(out=beta_sb, in_=beta_d)
            b2_sb = persist.tile([E, 1], F32)
            nc.sync.dma_start(out=b2_sb, in_=b2_d)

            with (
                tc.tile_pool(name="wld", bufs=1) as wld,
                tc.tile_pool(name="tp", bufs=2, space="PSUM") as tps,
            ):
                w1_sb = wld.tile([CC, C], F32)
                nc.sync.dma_start(out=w1_sb, in_=w1_d)
                w2_sb = wld.tile([E, CC * K * K], F32)
                nc.sync.dma_start(out=w2_sb, in_=w2_d)
                for ck in range(2):
                    pt = tps.tile([128, CC], F32, tag="w1t")
                    nc.tensor.transpose(pt, w1_sb[:, ck * 128:(ck + 1) * 128],
                                        ident[:CC, :CC])
                    nc.scalar.copy(out=w1T[:, ck, :], in_=pt)
                for t in range(K * K):
                    pt2 = tps.tile([CC, E], F32, tag="w2t")
                    src = _ap(w2_sb[:, :], [w2_sb[:, :].ap[0], [K * K, CC]],
                              extra_offset=t)
                    nc.tensor.transpose(pt2, src, ident[:E, :E])
                    nc.scalar.copy(out=w2T[:, t, :], in_=pt2)

            # ---------- x: load, cast to padded fp16, wide shifted copy ----
            x16 = persist.tile([128, 2, HP, WP16], F16)
            xbf = persist.tile([128, 2, HP, HP], F16)  # = x16 cols 1..66
            NLD = 32
            with tc.tile_pool(name="xfp", bufs=2) as xfp:
                for ck in range(2):
                    for r0 in range(0, H, NLD):
                        xf = xfp.tile([128, NLD, W], F32, tag="xload")
                        nc.sync.dma_start(
                            out=xf, in_=x_d[ck * 128:(ck + 1) * 128,
                                            r0:r0 + NLD, :])
                        nc.scalar.copy(
                            out=x16[:, ck, 1 + r0:1 + r0 + NLD, 2:2 + W],
                            in_=xf)
                for ck in range(2):
                    nc.vector.tensor_copy(out=x16[:, ck, 1:H + 1, 1:2],
                                          in_=x16[:, ck, 1:H + 1, 2:3])
                    nc.vector.tensor_copy(out=x16[:, ck, 1:H + 1, 66:67],
                                          in_=x16[:, ck, 1:H + 1, 65:66])
                    nc.vector.tensor_copy(out=x16[:, ck, 0:1, 1:67],
                                          in_=x16[:, ck, 1:2, 1:67])
                    nc.vector.tensor_copy(out=x16[:, ck, HP - 1:HP, 1:67],
                                          in_=x16[:, ck, HP - 2:HP - 1, 1:67])
                for ck in range(2):
                    eng = nc.scalar.copy if ck == 0 else nc.gpsimd.tensor_copy
                    eng(out=xbf[:, ck, :, :], in_=x16[:, ck, :, 1:1 + HP])

            # ---------- compressor + BN + encoder + softmax (scoped) -------
            kern_dr = dram.tile([E, H * W], F16)
            with tc.tile_pool(name="mid", bufs=1) as mid:
                comp = mid.tile([CC, HP, HP], F16)
                nc.vector.memset(comp[:, 0:1, :], 0.0)
                nc.vector.memset(comp[:, HP - 1:HP, :], 0.0)
                nc.vector.memset(comp[:, :, 0:1], 0.0)
                nc.vector.memset(comp[:, :, HP - 1:HP], 0.0)
                NCH = 8
                nchunks = H // NCH
                sum_p = mid.tile([CC, nchunks], F32)
                sq_p = mid.tile([CC, nchunks], F32)
                dump = mid.tile([CC, NCH * W], F16)
                with tc.tile_pool(name="cps", bufs=2, space="PSUM") as cps:
                    for hc in range(nchunks):
                        pc = cps.tile([CC, NCH, W], F32, tag="comp")
                        for ck in range(2):
                            nc.tensor.matmul(
                                pc, w1T[:, ck, :],
                                x16[:, ck, 1 + hc * NCH:1 + (hc + 1) * NCH,
                                    2:2 + W],
                                start=(ck == 0), stop=(ck == 1))
                        nc.scalar.activation(
                            out=comp[:, 1 + hc * NCH:1 + (hc + 1) * NCH,
                                     1:W + 1],
                            in_=pc, func=AF.Identity, bias=b1_sb, scale=1.0,
                            accum_out=sum_p[:, hc:hc + 1])
                    for hc in range(nchunks):
                        nc.scalar.activation(
                            out=dump.rearrange("p (a b) -> p a b", a=NCH),
                            in_=comp[:, 1 + hc * NCH:1 + (hc + 1) * NCH,
                                     1:W + 1],
                            func=AF.Square, accum_out=sq_p[:, hc:hc + 1])

                stats = mid.tile([CC, 2], F32)
                nc.vector.tensor_reduce(out=stats[:, 0:1], in_=sum_p,
                                        axis=AX.X, op=OP.add)
                nc.vector.tensor_reduce(out=stats[:, 1:2], in_=sq_p,
                                        axis=AX.X, op=OP.add)

                cc_in = dram.tile([CC, 2], F32)
                cc_out = dram.tile([CC, 2], F32)
                nc.gpsimd.dma_start(out=cc_in[:], in_=stats)
                nc.gpsimd.collective_compute(
                    "AllReduce", OP.add,
                    replica_groups=[list(range(NCORES))],
                    ins=[cc_in[:].opt()], outs=[cc_out[:].opt()])
                gstats = mid.tile([CC, 2], F32)
                nc.gpsimd.dma_start(out=gstats, in_=cc_out[:])

                mu = mid.tile([CC, 1], F32)
                var = mid.tile([CC, 1], F32)
                scl = mid.tile([CC, 1], F32)
                shf = mid.tile([CC, 1], F32)
                inv_n = 1.0 / (B * NPIX)
                nc.vector.tensor_scalar_mul(out=mu, in0=gstats[:, 0:1],
                                            scalar1=inv_n)
                nc.vector.tensor_scalar_mul(out=var, in0=gstats[:, 1:2],
                                            scalar1=inv_n)
                nc.vector.tensor_tensor(out=shf, in0=mu, in1=mu, op=OP.mult)
                nc.vector.tensor_tensor(out=var, in0=var, in1=shf,
                                        op=OP.subtract)
                eps_sb = mid.tile([CC, 1], F32)
                nc.vector.memset(eps_sb, EPS)
                nc.scalar.activation(out=var, in_=var, func=AF.Sqrt,
                                     bias=eps_sb, scale=1.0)
                nc.vector.reciprocal(out=var, in_=var)
                nc.vector.tensor_tensor(out=scl, in0=gamma_sb, in1=var,
                                        op=OP.mult)
                nc.vector.tensor_tensor(out=shf, in0=mu, in1=scl, op=OP.mult)
                nc.vector.tensor_tensor(out=shf, in0=beta_sb, in1=shf,
                                        op=OP.subtract)
                interior = comp[:, 1:H + 1, 1:W + 1]
                nc.scalar.activation(out=interior, in_=interior, func=AF.Relu,
                                     bias=shf, scale=scl)

                eexp = mid.tile([E, H, W], F16)
                with tc.tile_pool(name="eps", bufs=2, space="PSUM") as eps_pool:
                    for hc in range(nchunks):
                        pe = eps_pool.tile([E, NCH, W], F32, tag="enc")
                        for t in range(K * K):
                            ki, kj = t // K, t % K
                            nc.tensor.matmul(
                                pe, w2T[:, t, :],
                                comp[:, hc * NCH + ki:hc * NCH + ki + NCH,
                                     kj:kj + W],
                                start=(t == 0), stop=(t == K * K - 1))
                        nc.scalar.activation(
                            out=eexp[:, hc * NCH:(hc + 1) * NCH, :], in_=pe,
                            func=AF.Exp, bias=b2_sb, scale=1.0)

                zrec = mid.tile([E, W], F32)
                ee = eexp[:, :, :]
                ee_wh = _ap(ee, [ee.ap[0], [1, W], [W, H]])
                nc.vector.tensor_reduce(out=zrec, in_=ee_wh, axis=AX.X,
                                        op=OP.add)
                nc.vector.reciprocal(out=zrec, in_=zrec)
                zrec16 = mid.tile([E, W], F16)
                nc.vector.tensor_copy(out=zrec16, in_=zrec)
                kern = mid.tile([E, H, W], F16)
                zb = zrec16[:, :]
                nc.vector.tensor_tensor(
                    out=kern, in0=ee,
                    in1=_ap(zb, [zb.ap[0], [0, H], [1, W]]), op=OP.mult)
                nc.gpsimd.dma_start(out=kern_dr[:],
                                    in_=kern.rearrange("p a b -> p (a b)"))

            # ---------- reassembly (s-outer, full-row broadcasts) ----------
            with (
                tc.tile_pool(name="bc", bufs=9) as bcpool,
                tc.tile_pool(name="prod", bufs=4) as prodpool,
                tc.tile_pool(name="stg", bufs=3) as stgpool,
                tc.tile_pool(name="ops", bufs=2, space="PSUM") as pspool,
            ):
                for s in range(S * S):
                    bcs = []
                    for t in range(K * K):
                        ch = s * K * K + t
                        bc = bcpool.tile([128, H * W], F16, tag="bc",
                                         name=f"bc_{s}_{t}")
                        eng = (nc.gpsimd, nc.sync, nc.scalar)[t % 3]
                        eng.dma_start(
                            out=bc,
                            in_=_ap(kern_dr[ch:ch + 1, :],
                                    [[0, 128], [1, H * W]]))
                        bcs.append(bc)
                    for q in range(NQ):
                        h0 = q * HQ
                        psums = {}
                        for ck in range(2):
                            psums[ck] = pspool.tile(
                                [128, HQ * W], F32, tag=f"ps{ck}", bufs=2,
                                name=f"ps_{s}_{q}_{ck}")
                        for t in range(K * K):
                            ki, kj = t // K, t % K
                            in1 = _ap(bcs[t][:, :],
                                      [bcs[t][:, :].ap[0], [W, HQ], [1, W]],
                                      extra_offset=h0 * W)
                            for ck in range(2):
                                prod = prodpool.tile([128, HQ, W], F16)
                                if kj == 1:
                                    in0 = x16[:, ck, h0 + ki:h0 + ki + HQ,
                                              2:2 + W]
                                else:
                                    in0 = xbf[:, ck, h0 + ki:h0 + ki + HQ,
                                              kj:kj + W]
                                nc.vector.tensor_tensor(
                                    out=prod, in0=in0, in1=in1, op=OP.mult)
                                prod_f = prod.rearrange("p a b -> p (a b)")
                                for i in range(2):
                                    nc.tensor.matmul(
                                        psums[ck][:, i * 512:(i + 1) * 512],
                                        perm16,
                                        prod_f[:, i * 512:(i + 1) * 512],
                                        start=(t == 0), stop=(t == K * K - 1))
                        # evac: psum block kap = quadrant (hb, wb); value
                        # (p'=kap*32+chi, hl, w) -> olin[chi, 2*hl+hb, wb*64+w]
                        for ck in range(2):
                            olin = stgpool.tile([32, 2 * HQ, S * W], F32,
                                                tag="ol",
                                                name=f"olin_{s}_{q}_{ck}")
                            ob = olin[:, :, :]
                            for kap in range(4):
                                hb, wb = kap // 2, kap % 2
                                dst_view = _ap(
                                    ob, [ob.ap[0], [2 * S * W, HQ], [1, W]],
                                    extra_offset=hb * S * W + wb * W)
                                sl = psums[ck][kap * 32:(kap + 1) * 32, :]
                                src_view = _ap(sl,
                                               [sl.ap[0], [W, HQ], [1, W]])
                                nc.scalar.copy(out=dst_view, in_=src_view)
                            dst = bass.AP(
                                tensor=out_d.tensor,
                                offset=out_d.offset
                                + (s * 64 + ck * 32) * (S * H * S * W)
                                + (2 * h0) * (S * W),
                                ap=[[S * H * S * W, 32],
                                    [S * W, 2 * HQ],
                                    [1, S * W]])
                            nc.sync.dma_start(out=dst, in_=olin)
    nc.compile()
    return nc


_NC_CACHE = None


def _get_nc():
    global _NC_CACHE
    if _NC_CACHE is None:
        _NC_CACHE = build()
    return _NC_CACHE


def _perm_matrix():
    p = np.zeros((128, 128), dtype=np.float32)
    for pp in range(128):
        c = 4 * (pp % 32) + pp // 32
        p[c, pp] = 1.0
    return p


def _make_in_maps(inputs):
    x = np.ascontiguousarray(inputs["x"], dtype=np.float32)
    perm = _perm_matrix()
    in_maps = []
    for b in range(NCORES):
        in_maps.append({
            "perm": perm,
            "x": np.ascontiguousarray(x[b]),
            "w1": np.ascontiguousarray(inputs["w1"], dtype=np.float32),
            "b1": np.ascontiguousarray(np.asarray(inputs["b1"], dtype=np.float32).reshape(CC, 1)),
            "gamma": np.ascontiguousarray(np.asarray(inputs["gamma"], dtype=np.float32).reshape(CC, 1)),
            "beta": np.ascontiguousarray(np.asarray(inputs["beta"], dtype=np.float32).reshape(CC, 1)),
            "w2": np.ascontiguousarray(np.asarray(inputs["w2"], dtype=np.float32).reshape(E, CC * K * K)),
            "b2": np.ascontiguousarray(np.asarray(inputs["b2"], dtype=np.float32).reshape(E, 1)),
        })
    return in_maps


def kernel(x, w1, b1, gamma, beta, w2, b2, **kwargs):
    in_maps = _make_in_maps(dict(x=x, w1=w1, b1=b1, gamma=gamma, beta=beta,
                                 w2=w2, b2=b2))
    nc = _get_nc()
    res = run_bass_kernel_spmd(nc, in_maps, core_ids=list(range(NCORES)))
    return np.stack([res.results[b]["out"] for b in range(NCORES)], axis=0)


# revision 26
# speedup vs baseline: 1.1205x; 1.0404x over previous
"""CARAFE forward on 8 Trainium2 NeuronCores, data-parallel over batch.

Per core (1 sample):
  1. x loaded as 4 big contiguous DMAs, cast to fp16 padded layout x16
     ([*, 66, 68], center window 4B-aligned) plus one wide shifted copy
     xbf_w ([*, 66, 66]) whose kj=0 / kj=2 windows are both 4B-aligned,
     so every DVE multiply runs in 2x (16-bit packed) mode.
  2. 1x1 conv compressor (PE fp16); BN batch-stat sums fused into the psum
     evacuation (scalar accum_out), sumsq via a Square pass.
  3. BN stats AllReduce over the 8 cores (exact sync-BN), BN+ReLU applied.
  4. 3x3 encoder conv (PE, 9 taps psum-accumulated), fused bias+exp evac.
  5. softmax over H: strided reduce + reciprocal + fp16 normalize; kern
     written to DRAM so it can be partition-broadcast.
  6. reassembly, s-outer: per (s,t) ONE full-row broadcast [128, 4096]
     (8KB descriptors); DVE fp16 multiplies; tap accumulation on the PE
     via perm-matmuls (perm applies the faithful-to-source (s,c) reshape
     scramble: psum partition p' = (c%4)*32 + (c//4)%32, so each
     32-partition psum block is one output quadrant); scalar evacuates
     the 4 quadrant blocks into a [32, 2HQ, 128] staging tile that maps
     to a fully-contiguous 16KB-per-channel output DMA.
"""
import numpy as np

import concourse.bass as bass
import concourse.tile as tile
from concourse import bacc, mybir
from concourse.bass_utils import run_bass_kernel_spmd
from concourse.masks import make_identity

F32 = mybir.dt.float32
F16 = mybir.dt.float16
AX = mybir.AxisListType
OP = mybir.AluOpType
AF = mybir.ActivationFunctionType

B, C, H, W = 8, 256, 64, 64
CC = 64          # compressed channels
S = 2            # scale factor
K = 3            # kernel size
E = S * S * K * K  # 36 encoder channels
EPS = 1e-5
NCORES = 8
HP = H + 2       # padded rows
WP16 = 68        # x16 padded row width (col 0 unused, cols 1..66 = pad,x,pad)
NPIX = H * W
HQ = 16          # h rows per reassembly unit
NQ = H // HQ     # 4 units per (s)


def _ap(t, ap, extra_offset=0):
    return bass.AP(tensor=t.tensor, offset=t.offset + extra_offset, ap=ap)


def build():
    nc = bacc.Bacc("TRN2", target_bir_lowering=False, debug=False,
                   num_devices=NCORES)
    x_d = nc.dram_tensor("x", [C, H, W], F32, kind="ExternalInput").ap()
    w1_d = nc.dram_tensor("w1", [CC, C], F32, kind="ExternalInput").ap()
    b1_d = nc.dram_tensor("b1", [CC, 1], F32, kind="ExternalInput").ap()
    gamma_d = nc.dram_tensor("gamma", [CC, 1], F32, kind="ExternalInput").ap()
    beta_d = nc.dram_tensor("beta", [CC, 1], F32, kind="ExternalInput").ap()
    w2_d = nc.dram_tensor("w2", [E, CC * K * K], F32, kind="ExternalInput").ap()
    b2_d = nc.dram_tensor("b2", [E, 1], F32, kind="ExternalInput").ap()
    # perm[c, p'] = 1 iff c == 4*(p' % 32) + p' // 32
    perm_d = nc.dram_tensor("perm", [128, 128], F32, kind="ExternalInput").ap()
    out_d = nc.dram_tensor("out", [C, S * H, S * W], F32, kind="ExternalOutput").ap()

    with tile.TileContext(nc) as tc:
        with (
            tc.tile_pool(name="persist", bufs=1) as persist,
            tc.tile_pool(name="dram", bufs=1, space="DRAM") as dram,
        ):
            # ---------- constants & transposed weights ----------
            ident = persist.tile([128, 128], F32)
            make_identity(nc, ident)
            perm_sb = persist.tile([128, 128], F32)
            nc.sync.dma_start(out=perm_sb, in_=perm_d)
            perm16 = persist.tile([128, 128], F16)
            nc.scalar.copy(out=perm16, in_=perm_sb)

            w1T = persist.tile([128, 2, CC], F16)   # (c_part, chunk, o)
            w2T = persist.tile([CC, K * K, E], F16)  # (c, tap, e)
            b1_sb = persist.tile([CC, 1], F32)
            nc.sync.dma_start(out=b1_sb, in_=b1_d)
            gamma_sb = persist.tile([CC, 1], F32)
            nc.sync.dma_start(out=gamma_sb, in_=gamma_d)
            beta_sb = persist.tile([CC, 1], F32)
            nc.sync.dma_start(out=beta_sb, in_=beta_d)
            b2_sb = persist.tile([E, 1], F32)
            nc.sync.dma_start(out=b2_sb, in_=b2_d)

            with (
                tc.tile_pool(name="wld", bufs=1) as wld,
                tc.tile_pool(name="tp", bufs=2, space="PSUM") as tps,
            ):
                w1_sb = wld.tile([CC, C], F32)
                nc.sync.dma_start(out=w1_sb, in_=w1_d)
                w2_sb = wld.tile([E, CC * K * K], F32)
                nc.sync.dma_start(out=w2_sb, in_=w2_d)
                for ck in range(2):
                    pt = tps.tile([128, CC], F32, tag="w1t")
                    nc.tensor.transpose(pt, w1_sb[:, ck * 128:(ck + 1) * 128],
                                        ident[:CC, :CC])
                    nc.scalar.copy(out=w1T[:, ck, :], in_=pt)
                for t in range(K * K):
                    pt2 = tps.tile([CC, E], F32, tag="w2t")
                    src = _ap(w2_sb[:, :], [w2_sb[:, :].ap[0], [K * K, CC]],
                              extra_offset=t)
                    nc.tensor.transpose(pt2, src, ident[:E, :E])
                    nc.scalar.copy(out=w2T[:, t, :], in_=pt2)

            # ---------- x: load, cast to padded fp16, wide shifted copy ----
            x16 = persist.tile([128, 2, HP, WP16], F16)
            xbf = persist.tile([128, 2, HP, HP], F16)  # = x16 cols 1..66
            NLD = 32
            with tc.tile_pool(name="xfp", bufs=2) as xfp:
                for ck in range(2):
                    for r0 in range(0, H, NLD):
                        xf = xfp.tile([128, NLD, W], F32, tag="xload")
                        nc.sync.dma_start(
                            out=xf, in_=x_d[ck * 128:(ck + 1) * 128,
                                            r0:r0 + NLD, :])
                        nc.scalar.copy(
                            out=x16[:, ck, 1 + r0:1 + r0 + NLD, 2:2 + W],
                            in_=xf)
                for ck in range(2):
                    nc.vector.tensor_copy(out=x16[:, ck, 1:H + 1, 1:2],
                                          in_=x16[:, ck, 1:H + 1, 2:3])
                    nc.vector.tensor_copy(out=x16[:, ck, 1:H + 1, 66:67],
                                          in_=x16[:, ck, 1:H + 1, 65:66])
                    nc.vector.tensor_copy(out=x16[:, ck, 0:1, 1:67],
                                          in_=x16[:, ck, 1:2, 1:67])
                    nc.vector.tensor_copy(out=x16[:, ck, HP - 1:HP, 1:67],
                                          in_=x16[:, ck, HP - 2:HP - 1, 1:67])
                for ck in range(2):
                    eng = nc.scalar.copy if ck == 0 else nc.gpsimd.tensor_copy
                    eng(out=xbf[:, ck, :, :], in_=x16[:, ck, :, 1:1 + HP])

            # ---------- compressor + BN + encoder + softmax (scoped) -------
            kern_dr = dram.tile([E, H * W], F16)
            with tc.tile_pool(name="mid", bufs=1) as mid:
                comp = mid.tile([CC, HP, HP], F16)
                nc.vector.memset(comp[:, 0:1, :], 0.0)
                nc.vector.memset(comp[:, HP - 1:HP, :], 0.0)
                nc.vector.memset(comp[:, :, 0:1], 0.0)
                nc.vector.memset(comp[:, :, HP - 1:HP], 0.0)
                NCH = 8
                nchunks = H // NCH
                sum_p = mid.tile([CC, nchunks], F32)
                sq_p = mid.tile([CC, nchunks], F32)
                dump = mid.tile([CC, NCH * W], F16)
                with tc.tile_pool(name="cps", bufs=2, space="PSUM") as cps:
                    for hc in range(nchunks):
                        pc = cps.tile([CC, NCH, W], F32, tag="comp")
                        for ck in range(2):
                            nc.tensor.matmul(
                                pc, w1T[:, ck, :],
                                x16[:, ck, 1 + hc * NCH:1 + (hc + 1) * NCH,
                                    2:2 + W],
                                start=(ck == 0), stop=(ck == 1))
                        nc.scalar.activation(
                            out=comp[:, 1 + hc * NCH:1 + (hc + 1) * NCH,
                                     1:W + 1],
                            in_=pc, func=AF.Identity, bias=b1_sb, scale=1.0,
                            accum_out=sum_p[:, hc:hc + 1])
                    for hc in range(nchunks):
                        nc.scalar.activation(
                            out=dump.rearrange("p (a b) -> p a b", a=NCH),
                            in_=comp[:, 1 + hc * NCH:1 + (hc + 1) * NCH,
                                     1:W + 1],
                            func=AF.Square, accum_out=sq_p[:, hc:hc + 1])

                stats = mid.tile([CC, 2], F32)
                nc.vector.tensor_reduce(out=stats[:, 0:1], in_=sum_p,
                                        axis=AX.X, op=OP.add)
                nc.vector.tensor_reduce(out=stats[:, 1:2], in_=sq_p,
                                        axis=AX.X, op=OP.add)

                cc_in = dram.tile([CC, 2], F32)
                cc_out = dram.tile([CC, 2], F32)
                nc.gpsimd.dma_start(out=cc_in[:], in_=stats)
                nc.gpsimd.collective_compute(
                    "AllReduce", OP.add,
                    replica_groups=[list(range(NCORES))],
                    ins=[cc_in[:].opt()], outs=[cc_out[:].opt()])
                gstats = mid.tile([CC, 2], F32)
                nc.gpsimd.dma_start(out=gstats, in_=cc_out[:])

                mu = mid.tile([CC, 1], F32)
                var = mid.tile([CC, 1], F32)
                scl = mid.tile([CC, 1], F32)
                shf = mid.tile([CC, 1], F32)
                inv_n = 1.0 / (B * NPIX)
                nc.vector.tensor_scalar_mul(out=mu, in0=gstats[:, 0:1],
                                            scalar1=inv_n)
                nc.vector.tensor_scalar_mul(out=var, in0=gstats[:, 1:2],
                                            scalar1=inv_n)
                nc.vector.tensor_tensor(out=shf, in0=mu, in1=mu, op=OP.mult)
                nc.vector.tensor_tensor(out=var, in0=var, in1=shf,
                                        op=OP.subtract)
                eps_sb = mid.tile([CC, 1], F32)
                nc.vector.memset(eps_sb, EPS)
                nc.scalar.activation(out=var, in_=var, func=AF.Sqrt,
                                     bias=eps_sb, scale=1.0)
                nc.vector.reciprocal(out=var, in_=var)
                nc.vector.tensor_tensor(out=scl, in0=gamma_sb, in1=var,
                                        op=OP.mult)
                nc.vector.tensor_tensor(out=shf, in0=mu, in1=scl, op=OP.mult)
                nc.vector.tensor_tensor(out=shf, in0=beta_sb, in1=shf,
                                        op=OP.subtract)
                interior = comp[:, 1:H + 1, 1:W + 1]
                nc.scalar.activation(out=interior, in_=interior, func=AF.Relu,
                                     bias=shf, scale=scl)

                eexp = mid.tile([E, H, W], F16)
                with tc.tile_pool(name="eps", bufs=2, space="PSUM") as eps_pool:
                    for hc in range(nchunks):
                        pe = eps_pool.tile([E, NCH, W], F32, tag="enc")
                        for t in range(K * K):
                            ki, kj = t // K, t % K
                            nc.tensor.matmul(
                                pe, w2T[:, t, :],
                                comp[:, hc * NCH + ki:hc * NCH + ki + NCH,
                                     kj:kj + W],
                                start=(t == 0), stop=(t == K * K - 1))
                        nc.scalar.activation(
                            out=eexp[:, hc * NCH:(hc + 1) * NCH, :], in_=pe,
                            func=AF.Exp, bias=b2_sb, scale=1.0)

                zrec = mid.tile([E, W], F32)
                ee = eexp[:, :, :]
                ee_wh = _ap(ee, [ee.ap[0], [1, W], [W, H]])
                nc.vector.tensor_reduce(out=zrec, in_=ee_wh, axis=AX.X,
                                        op=OP.add)
                nc.vector.reciprocal(out=zrec, in_=zrec)
                zrec16 = mid.tile([E, W], F16)
                nc.vector.tensor_copy(out=zrec16, in_=zrec)
                kern = mid.tile([E, H, W], F16)
                zb = zrec16[:, :]
                nc.vector.tensor_tensor(
                    out=kern, in0=ee,
                    in1=_ap(zb, [zb.ap[0], [0, H], [1, W]]), op=OP.mult)
                nc.gpsimd.dma_start(out=kern_dr[:],
                                    in_=kern.rearrange("p a b -> p (a b)"))

            # ---------- reassembly (s-outer, full-row broadcasts) ----------
            with (
                tc.tile_pool(name="bc", bufs=10) as bcpool,
                tc.tile_pool(name="prod", bufs=4) as prodpool,
                tc.tile_pool(name="stg", bufs=3) as stgpool,
                tc.tile_pool(name="ops", bufs=2, space="PSUM") as pspool,
            ):
                for s in range(S * S):
                    bcs = []
                    for t in range(K * K):
                        ch = s * K * K + t
                        bc = bcpool.tile([128, H * W], F16, tag="bc",
                                         name=f"bc_{s}_{t}")
                        eng = (nc.gpsimd, nc.gpsimd, nc.scalar)[t % 3]
                        eng.dma_start(
                            out=bc,
                            in_=_ap(kern_dr[ch:ch + 1, :],
                                    [[0, 128], [1, H * W]]))
                        bcs.append(bc)
                    for q in range(NQ):
                        h0 = q * HQ
                        psums = {}
                        for ck in range(2):
                            psums[ck] = pspool.tile(
                                [128, HQ * W], F32, tag=f"ps{ck}", bufs=2,
                                name=f"ps_{s}_{q}_{ck}")
                        for t in range(K * K):
                            ki, kj = t // K, t % K
                            in1 = _ap(bcs[t][:, :],
                                      [bcs[t][:, :].ap[0], [W, HQ], [1, W]],
                                      extra_offset=h0 * W)
                            for ck in range(2):
                                prod = prodpool.tile([128, HQ, W], F16)
                                if kj == 1:
                                    in0 = x16[:, ck, h0 + ki:h0 + ki + HQ,
                                              2:2 + W]
                                else:
                                    in0 = xbf[:, ck, h0 + ki:h0 + ki + HQ,
                                              kj:kj + W]
                                nc.vector.tensor_tensor(
                                    out=prod, in0=in0, in1=in1, op=OP.mult)
                                prod_f = prod.rearrange("p a b -> p (a b)")
                                for i in range(2):
                                    nc.tensor.matmul(
                                        psums[ck][:, i * 512:(i + 1) * 512],
                                        perm16,
                                        prod_f[:, i * 512:(i + 1) * 512],
                                        start=(t == 0), stop=(t == K * K - 1))
                        # evac: psum block kap = quadrant (hb, wb); value
                        # (p'=kap*32+chi, hl, w) -> olin[chi, 2*hl+hb, wb*64+w]
                        for ck in range(2):
                            olin = stgpool.tile([32, 2 * HQ, S * W], F32,
                                                tag="ol",
                                                name=f"olin_{s}_{q}_{ck}")
                            ob = olin[:, :, :]
                            for kap in range(4):
                                hb, wb = kap // 2, kap % 2
                                dst_view = _ap(
                                    ob, [ob.ap[0], [2 * S * W, HQ], [1, W]],
                                    extra_offset=hb * S * W + wb * W)
                                sl = psums[ck][kap * 32:(kap + 1) * 32, :]
                                src_view = _ap(sl,
                                               [sl.ap[0], [W, HQ], [1, W]])
                                nc.scalar.copy(out=dst_view, in_=src_view)
                            dst = bass.AP(
                                tensor=out_d.tensor,
                                offset=out_d.offset
                                + (s * 64 + ck * 32) * (S * H * S * W)
                                + (2 * h0) * (S * W),
                                ap=[[S * H * S * W, 32],
                                    [S * W, 2 * HQ],
                                    [1, S * W]])
                            nc.sync.dma_start(out=dst, in_=olin)
    nc.compile()
    return nc


_NC_CACHE = None


def _get_nc():
    global _NC_CACHE
    if _NC_CACHE is None:
        _NC_CACHE = build()
    return _NC_CACHE


def _perm_matrix():
    p = np.zeros((128, 128), dtype=np.float32)
    for pp in range(128):
        c = 4 * (pp % 32) + pp // 32
        p[c, pp] = 1.0
    return p


def _make_in_maps(inputs):
    x = np.ascontiguousarray(inputs["x"], dtype=np.float32)
    perm = _perm_matrix()
    in_maps = []
    for b in range(NCORES):
        in_maps.append({
            "perm": perm,
            "x": np.ascontiguousarray(x[b]),
            "w1": np.ascontiguousarray(inputs["w1"], dtype=np.float32),
            "b1": np.ascontiguousarray(np.asarray(inputs["b1"], dtype=np.float32).reshape(CC, 1)),
            "gamma": np.ascontiguousarray(np.asarray(inputs["gamma"], dtype=np.float32).reshape(CC, 1)),
            "beta": np.ascontiguousarray(np.asarray(inputs["beta"], dtype=np.float32).reshape(CC, 1)),
            "w2": np.ascontiguousarray(np.asarray(inputs["w2"], dtype=np.float32).reshape(E, CC * K * K)),
            "b2": np.ascontiguousarray(np.asarray(inputs["b2"], dtype=np.float32).reshape(E, 1)),
        })
    return in_maps


def kernel(x, w1, b1, gamma, beta, w2, b2, **kwargs):
    in_maps = _make_in_maps(dict(x=x, w1=w1, b1=b1, gamma=gamma, beta=beta,
                                 w2=w2, b2=b2))
    nc = _get_nc()
    res = run_bass_kernel_spmd(nc, in_maps, core_ids=list(range(NCORES)))
    return np.stack([res.results[b]["out"] for b in range(NCORES)], axis=0)


# revision 27
# speedup vs baseline: 1.1409x; 1.0182x over previous
"""CARAFE forward on 8 Trainium2 NeuronCores, data-parallel over batch.

Per core (1 sample):
  1. x loaded as 4 big contiguous DMAs, cast to fp16 padded layout x16
     ([*, 66, 68], center window 4B-aligned) plus one wide shifted copy
     xbf_w ([*, 66, 66]) whose kj=0 / kj=2 windows are both 4B-aligned,
     so every DVE multiply runs in 2x (16-bit packed) mode.
  2. 1x1 conv compressor (PE fp16); BN batch-stat sums fused into the psum
     evacuation (scalar accum_out), sumsq via a Square pass.
  3. BN stats AllReduce over the 8 cores (exact sync-BN), BN+ReLU applied.
  4. 3x3 encoder conv (PE, 9 taps psum-accumulated), fused bias+exp evac.
  5. softmax over H: strided reduce + reciprocal + fp16 normalize; kern
     written to DRAM so it can be partition-broadcast.
  6. reassembly, s-outer: per (s,t) ONE full-row broadcast [128, 4096]
     (8KB descriptors); DVE fp16 multiplies; tap accumulation on the PE
     via perm-matmuls (perm applies the faithful-to-source (s,c) reshape
     scramble: psum partition p' = (c%4)*32 + (c//4)%32, so each
     32-partition psum block is one output quadrant); scalar evacuates
     the 4 quadrant blocks into a [32, 2HQ, 128] staging tile that maps
     to a fully-contiguous 16KB-per-channel output DMA.
"""
import numpy as np

import concourse.bass as bass
import concourse.tile as tile
from concourse import bacc, mybir
from concourse.bass_utils import run_bass_kernel_spmd
from concourse.masks import make_identity

F32 = mybir.dt.float32
F16 = mybir.dt.float16
AX = mybir.AxisListType
OP = mybir.AluOpType
AF = mybir.ActivationFunctionType

B, C, H, W = 8, 256, 64, 64
CC = 64          # compressed channels
S = 2            # scale factor
K = 3            # kernel size
E = S * S * K * K  # 36 encoder channels
EPS = 1e-5
NCORES = 8
HP = H + 2       # padded rows
WP16 = 68        # x16 padded row width (col 0 unused, cols 1..66 = pad,x,pad)
NPIX = H * W
HQ = 16          # h rows per reassembly unit
NQ = H // HQ     # 4 units per (s)


def _ap(t, ap, extra_offset=0):
    return bass.AP(tensor=t.tensor, offset=t.offset + extra_offset, ap=ap)


def build():
    nc = bacc.Bacc("TRN2", target_bir_lowering=False, debug=False,
                   num_devices=NCORES)
    x_d = nc.dram_tensor("x", [C, H, W], F32, kind="ExternalInput").ap()
    w1_d = nc.dram_tensor("w1", [CC, C], F32, kind="ExternalInput").ap()
    b1_d = nc.dram_tensor("b1", [CC, 1], F32, kind="ExternalInput").ap()
    gamma_d = nc.dram_tensor("gamma", [CC, 1], F32, kind="ExternalInput").ap()
    beta_d = nc.dram_tensor("beta", [CC, 1], F32, kind="ExternalInput").ap()
    w2_d = nc.dram_tensor("w2", [E, CC * K * K], F32, kind="ExternalInput").ap()
    b2_d = nc.dram_tensor("b2", [E, 1], F32, kind="ExternalInput").ap()
    # perm[c, p'] = 1 iff c == 4*(p' % 32) + p' // 32
    perm_d = nc.dram_tensor("perm", [128, 128], F32, kind="ExternalInput").ap()
    out_d = nc.dram_tensor("out", [C, S * H, S * W], F32, kind="ExternalOutput").ap()

    with tile.TileContext(nc) as tc:
        with (
            tc.tile_pool(name="persist", bufs=1) as persist,
            tc.tile_pool(name="dram", bufs=1, space="DRAM") as dram,
        ):
            # ---------- constants & transposed weights ----------
            ident = persist.tile([128, 128], F32)
            make_identity(nc, ident)
            perm_sb = persist.tile([128, 128], F32)
            nc.sync.dma_start(out=perm_sb, in_=perm_d)
            perm16 = persist.tile([128, 128], F16)
            nc.scalar.copy(out=perm16, in_=perm_sb)

            w1T = persist.tile([128, 2, CC], F16)   # (c_part, chunk, o)
            w2T = persist.tile([CC, K * K, E], F16)  # (c, tap, e)
            b1_sb = persist.tile([CC, 1], F32)
            nc.sync.dma_start(out=b1_sb, in_=b1_d)
            gamma_sb = persist.tile([CC, 1], F32)
            nc.sync.dma_start(out=gamma_sb, in_=gamma_d)
            beta_sb = persist.tile([CC, 1], F32)
            nc.sync.dma_start(out=beta_sb, in_=beta_d)
            b2_sb = persist.tile([E, 1], F32)
            nc.sync.dma_start(out=b2_sb, in_=b2_d)

            with (
                tc.tile_pool(name="wld", bufs=1) as wld,
                tc.tile_pool(name="tp", bufs=2, space="PSUM") as tps,
            ):
                w1_sb = wld.tile([CC, C], F32)
                nc.sync.dma_start(out=w1_sb, in_=w1_d)
                w2_sb = wld.tile([E, CC * K * K], F32)
                nc.sync.dma_start(out=w2_sb, in_=w2_d)
                for ck in range(2):
                    pt = tps.tile([128, CC], F32, tag="w1t")
                    nc.tensor.transpose(pt, w1_sb[:, ck * 128:(ck + 1) * 128],
                                        ident[:CC, :CC])
                    nc.scalar.copy(out=w1T[:, ck, :], in_=pt)
                for t in range(K * K):
                    pt2 = tps.tile([CC, E], F32, tag="w2t")
                    src = _ap(w2_sb[:, :], [w2_sb[:, :].ap[0], [K * K, CC]],
                              extra_offset=t)
                    nc.tensor.transpose(pt2, src, ident[:E, :E])
                    nc.scalar.copy(out=w2T[:, t, :], in_=pt2)

            # ---------- x: load, cast to padded fp16, wide shifted copy ----
            x16 = persist.tile([128, 2, HP, WP16], F16)
            xbf = persist.tile([128, 2, HP, HP], F16)  # = x16 cols 1..66
            NLD = 32
            with tc.tile_pool(name="xfp", bufs=2) as xfp:
                for ck in range(2):
                    for r0 in range(0, H, NLD):
                        xf = xfp.tile([128, NLD, W], F32, tag="xload")
                        nc.sync.dma_start(
                            out=xf, in_=x_d[ck * 128:(ck + 1) * 128,
                                            r0:r0 + NLD, :])
                        nc.scalar.copy(
                            out=x16[:, ck, 1 + r0:1 + r0 + NLD, 2:2 + W],
                            in_=xf)
                for ck in range(2):
                    nc.vector.tensor_copy(out=x16[:, ck, 1:H + 1, 1:2],
                                          in_=x16[:, ck, 1:H + 1, 2:3])
                    nc.vector.tensor_copy(out=x16[:, ck, 1:H + 1, 66:67],
                                          in_=x16[:, ck, 1:H + 1, 65:66])
                    nc.vector.tensor_copy(out=x16[:, ck, 0:1, 1:67],
                                          in_=x16[:, ck, 1:2, 1:67])
                    nc.vector.tensor_copy(out=x16[:, ck, HP - 1:HP, 1:67],
                                          in_=x16[:, ck, HP - 2:HP - 1, 1:67])
                for ck in range(2):
                    eng = nc.scalar.copy if ck == 0 else nc.gpsimd.tensor_copy
                    eng(out=xbf[:, ck, :, :], in_=x16[:, ck, :, 1:1 + HP])

            # ---------- compressor + BN + encoder + softmax (scoped) -------
            kern_dr = dram.tile([E, H * W], F16)
            with tc.tile_pool(name="mid", bufs=1) as mid:
                comp = mid.tile([CC, HP, HP], F16)
                nc.vector.memset(comp[:, 0:1, :], 0.0)
                nc.vector.memset(comp[:, HP - 1:HP, :], 0.0)
                nc.vector.memset(comp[:, :, 0:1], 0.0)
                nc.vector.memset(comp[:, :, HP - 1:HP], 0.0)
                NCH = 8
                nchunks = H // NCH
                sum_p = mid.tile([CC, nchunks], F32)
                sq_p = mid.tile([CC, nchunks], F32)
                dump = mid.tile([CC, NCH * W], F16)
                with tc.tile_pool(name="cps", bufs=2, space="PSUM") as cps:
                    for hc in range(nchunks):
                        pc = cps.tile([CC, NCH, W], F32, tag="comp")
                        for ck in range(2):
                            nc.tensor.matmul(
                                pc, w1T[:, ck, :],
                                x16[:, ck, 1 + hc * NCH:1 + (hc + 1) * NCH,
                                    2:2 + W],
                                start=(ck == 0), stop=(ck == 1))
                        nc.scalar.activation(
                            out=comp[:, 1 + hc * NCH:1 + (hc + 1) * NCH,
                                     1:W + 1],
                            in_=pc, func=AF.Identity, bias=b1_sb, scale=1.0,
                            accum_out=sum_p[:, hc:hc + 1])
                    for hc in range(nchunks):
                        nc.scalar.activation(
                            out=dump.rearrange("p (a b) -> p a b", a=NCH),
                            in_=comp[:, 1 + hc * NCH:1 + (hc + 1) * NCH,
                                     1:W + 1],
                            func=AF.Square, accum_out=sq_p[:, hc:hc + 1])

                stats = mid.tile([CC, 2], F32)
                nc.vector.tensor_reduce(out=stats[:, 0:1], in_=sum_p,
                                        axis=AX.X, op=OP.add)
                nc.vector.tensor_reduce(out=stats[:, 1:2], in_=sq_p,
                                        axis=AX.X, op=OP.add)

                cc_in = dram.tile([CC, 2], F32)
                cc_out = dram.tile([CC, 2], F32)
                nc.gpsimd.dma_start(out=cc_in[:], in_=stats)
                nc.gpsimd.collective_compute(
                    "AllReduce", OP.add,
                    replica_groups=[list(range(NCORES))],
                    ins=[cc_in[:].opt()], outs=[cc_out[:].opt()])
                gstats = mid.tile([CC, 2], F32)
                nc.gpsimd.dma_start(out=gstats, in_=cc_out[:])

                mu = mid.tile([CC, 1], F32)
                var = mid.tile([CC, 1], F32)
                scl = mid.tile([CC, 1], F32)
                shf = mid.tile([CC, 1], F32)
                inv_n = 1.0 / (B * NPIX)
                nc.vector.tensor_scalar_mul(out=mu, in0=gstats[:, 0:1],
                                            scalar1=inv_n)
                nc.vector.tensor_scalar_mul(out=var, in0=gstats[:, 1:2],
                                            scalar1=inv_n)
                nc.vector.tensor_tensor(out=shf, in0=mu, in1=mu, op=OP.mult)
                nc.vector.tensor_tensor(out=var, in0=var, in1=shf,
                                        op=OP.subtract)
                eps_sb = mid.tile([CC, 1], F32)
                nc.vector.memset(eps_sb, EPS)
                nc.scalar.activation(out=var, in_=var, func=AF.Sqrt,
                                     bias=eps_sb, scale=1.0)
                nc.vector.reciprocal(out=var, in_=var)
                nc.vector.tensor_tensor(out=scl, in0=gamma_sb, in1=var,
                                        op=OP.mult)
                nc.vector.tensor_tensor(out=shf, in0=mu, in1=scl, op=OP.mult)
                nc.vector.tensor_tensor(out=shf, in0=beta_sb, in1=shf,
                                        op=OP.subtract)
                interior = comp[:, 1:H + 1, 1:W + 1]
                nc.scalar.activation(out=interior, in_=interior, func=AF.Relu,
                                     bias=shf, scale=scl)

                eexp = mid.tile([E, H, W], F16)
                with tc.tile_pool(name="eps", bufs=2, space="PSUM") as eps_pool:
                    for hc in range(nchunks):
                        pe = eps_pool.tile([E, NCH, W], F32, tag="enc")
                        for t in range(K * K):
                            ki, kj = t // K, t % K
                            nc.tensor.matmul(
                                pe, w2T[:, t, :],
                                comp[:, hc * NCH + ki:hc * NCH + ki + NCH,
                                     kj:kj + W],
                                start=(t == 0), stop=(t == K * K - 1))
                        nc.scalar.activation(
                            out=eexp[:, hc * NCH:(hc + 1) * NCH, :], in_=pe,
                            func=AF.Exp, bias=b2_sb, scale=1.0)

                zrec = mid.tile([E, W], F32)
                ee = eexp[:, :, :]
                ee_wh = _ap(ee, [ee.ap[0], [1, W], [W, H]])
                nc.vector.tensor_reduce(out=zrec, in_=ee_wh, axis=AX.X,
                                        op=OP.add)
                nc.vector.reciprocal(out=zrec, in_=zrec)
                zrec16 = mid.tile([E, W], F16)
                nc.vector.tensor_copy(out=zrec16, in_=zrec)
                kern = mid.tile([E, H, W], F16)
                zb = zrec16[:, :]
                nc.vector.tensor_tensor(
                    out=kern, in0=ee,
                    in1=_ap(zb, [zb.ap[0], [0, H], [1, W]]), op=OP.mult)
                nc.gpsimd.dma_start(out=kern_dr[:],
                                    in_=kern.rearrange("p a b -> p (a b)"))

            # ---------- reassembly (s-outer, full-row broadcasts) ----------
            with (
                tc.tile_pool(name="bc", bufs=12) as bcpool,
                tc.tile_pool(name="prod", bufs=4) as prodpool,
                tc.tile_pool(name="stg", bufs=3) as stgpool,
                tc.tile_pool(name="ops", bufs=2, space="PSUM") as pspool,
            ):
                for s in range(S * S):
                    bcs = []
                    for t in range(K * K):
                        ch = s * K * K + t
                        bc = bcpool.tile([128, H * W], F16, tag="bc",
                                         name=f"bc_{s}_{t}")
                        eng = (nc.gpsimd, nc.gpsimd, nc.scalar)[t % 3]
                        eng.dma_start(
                            out=bc,
                            in_=_ap(kern_dr[ch:ch + 1, :],
                                    [[0, 128], [1, H * W]]))
                        bcs.append(bc)
                    for q in range(NQ):
                        h0 = q * HQ
                        psums = {}
                        for ck in range(2):
                            psums[ck] = pspool.tile(
                                [128, HQ * W], F32, tag=f"ps{ck}", bufs=2,
                                name=f"ps_{s}_{q}_{ck}")
                        for t in range(K * K):
                            ki, kj = t // K, t % K
                            in1 = _ap(bcs[t][:, :],
                                      [bcs[t][:, :].ap[0], [W, HQ], [1, W]],
                                      extra_offset=h0 * W)
                            for ck in range(2):
                                prod = prodpool.tile([128, HQ, W], F16)
                                if kj == 1:
                                    in0 = x16[:, ck, h0 + ki:h0 + ki + HQ,
                                              2:2 + W]
                                else:
                                    in0 = xbf[:, ck, h0 + ki:h0 + ki + HQ,
                                              kj:kj + W]
                                nc.vector.tensor_tensor(
                                    out=prod, in0=in0, in1=in1, op=OP.mult)
                                prod_f = prod.rearrange("p a b -> p (a b)")
                                for i in range(2):
                                    nc.tensor.matmul(
                                        psums[ck][:, i * 512:(i + 1) * 512],
                                        perm16,
                                        prod_f[:, i * 512:(i + 1) * 512],
                                        start=(t == 0), stop=(t == K * K - 1))
                        # evac: psum block kap = quadrant (hb, wb); value
                        # (p'=kap*32+chi, hl, w) -> olin[chi, 2*hl+hb, wb*64+w]
                        for ck in range(2):
                            olin = stgpool.tile([32, 2 * HQ, S * W], F32,
                                                tag="ol",
                                                name=f"olin_{s}_{q}_{ck}")
                            ob = olin[:, :, :]
                            for kap in range(4):
                                hb, wb = kap // 2, kap % 2
                                dst_view = _ap(
                                    ob, [ob.ap[0], [2 * S * W, HQ], [1, W]],
                                    extra_offset=hb * S * W + wb * W)
                                sl = psums[ck][kap * 32:(kap + 1) * 32, :]
                                src_view = _ap(sl,
                                               [sl.ap[0], [W, HQ], [1, W]])
                                nc.scalar.copy(out=dst_view, in_=src_view)
                            dst = bass.AP(
                                tensor=out_d.tensor,
                                offset=out_d.offset
                                + (s * 64 + ck * 32) * (S * H * S * W)
                                + (2 * h0) * (S * W),
                                ap=[[S * H * S * W, 32],
                                    [S * W, 2 * HQ],
                                    [1, S * W]])
                            nc.sync.dma_start(out=dst, in_=olin)
    nc.compile()
    return nc


_NC_CACHE = None


def _get_nc():
    global _NC_CACHE
    if _NC_CACHE is None:
        _NC_CACHE = build()
    return _NC_CACHE


def _perm_matrix():
    p = np.zeros((128, 128), dtype=np.float32)
    for pp in range(128):
        c = 4 * (pp % 32) + pp // 32
        p[c, pp] = 1.0
    return p


def _make_in_maps(inputs):
    x = np.ascontiguousarray(inputs["x"], dtype=np.float32)
    perm = _perm_matrix()
    in_maps = []
    for b in range(NCORES):
        in_maps.append({
            "perm": perm,
            "x": np.ascontiguousarray(x[b]),
            "w1": np.ascontiguousarray(inputs["w1"], dtype=np.float32),
            "b1": np.ascontiguousarray(np.asarray(inputs["b1"], dtype=np.float32).reshape(CC, 1)),
            "gamma": np.ascontiguousarray(np.asarray(inputs["gamma"], dtype=np.float32).reshape(CC, 1)),
            "beta": np.ascontiguousarray(np.asarray(inputs["beta"], dtype=np.float32).reshape(CC, 1)),
            "w2": np.ascontiguousarray(np.asarray(inputs["w2"], dtype=np.float32).reshape(E, CC * K * K)),
            "b2": np.ascontiguousarray(np.asarray(inputs["b2"], dtype=np.float32).reshape(E, 1)),
        })
    return in_maps


def kernel(x, w1, b1, gamma, beta, w2, b2, **kwargs):
    in_maps = _make_in_maps(dict(x=x, w1=w1, b1=b1, gamma=gamma, beta=beta,
                                 w2=w2, b2=b2))
    nc = _get_nc()
    res = run_bass_kernel_spmd(nc, in_maps, core_ids=list(range(NCORES)))
    return np.stack([res.results[b]["out"] for b in range(NCORES)], axis=0)
